# revision 2
# baseline (speedup 1.0000x reference)
"""KNN classifier kernel for Trainium2 (8 NeuronCores, Bass/Tile).

Problem (nn_KNNClassifier): given queries x [4096, 512], train bank
x_train [65536, 512], labels y_train [65536] (100 classes), compute for
each query the top-200 neighbors by dot-product similarity, weight them
by exp(sim/0.1), accumulate per-class scores, and return the descending
argsort of class scores -> int32 [4096, 100].

Device strategy (sharding_hint: shard train bank over N across 8 cores):
  - Each core holds an N-shard of x_train^T (8192 columns) and the full
    set of queries. It computes sim = x @ x_train_shard^T via float32r
    matmuls (full PE rate), and for every 256-column chunk of the local
    sim rows extracts the top-8 values (DVE max8) and their positions
    (DVE max_index). 32 chunks/core -> 256 candidates per (query, core).
  - Host gathers 8*256 = 2048 candidates per query: provably a superset
    of the global top-200 unless some chunk had >8 entries above the
    candidate threshold, which is detected exactly (chunk 8th-max >=
    threshold - slack) and handled by an exact per-query fallback.
  - float32r is TF32-like (measured |err| <= ~0.02 at K=512); all
    candidates within SLACK of the top-200 threshold are recomputed
    exactly on host, so the selected top-200 set matches fp32 semantics.
  - Final per-class accumulation mimics the reference exactly (fp32 exp
    -> scatter-add -> stable argsort of negated scores).
"""

import os
import sys

for _p in ("/opt/trn_rl_repo",):
    if _p not in sys.path and os.path.isdir(_p):
        sys.path.insert(0, _p)

import numpy as np

import concourse.mybir as mybir
import concourse.tile as tile
from concourse import bacc
from concourse.bass_utils import run_bass_kernel_spmd

# Problem shapes (hardcoded per spec)
B, N, D = 4096, 65536, 512
NUM_CLASSES = 100
KNN_K = 200
KNN_T = 0.1
NCORES = 8

NLOC = N // NCORES  # 8192 train columns per core
QTRS = 4  # stream x_train^T shard in quarters
NQ = NLOC // QTRS  # 2048 columns per quarter
NTILE = 512  # matmul moving free dim
NT = NQ // NTILE  # 4 n-tiles per quarter
KT = D // 128  # 4 contraction tiles
QB = B // 128  # 32 query blocks of 128
CHUNK = 256  # top-8 extraction chunk width
CPQ = NQ // CHUNK  # 8 chunks per quarter
CANDS = (NLOC // CHUNK) * 8  # 256 candidates per (query, core)

SLACK = 0.1  # exact-recompute band around the top-200 threshold

_CACHE = {}
LAST_INFO = {}


def _build_program():
    """Build + compile the per-core Bass program (same program on all cores)."""
    nc = bacc.Bacc(
        "TRN2", target_bir_lowering=False, debug=False, num_devices=NCORES
    )
    f32 = mybir.dt.float32
    f32r = mybir.dt.float32r
    u32 = mybir.dt.uint32

    xT_d = nc.dram_tensor("xT", (D, B), f32r, kind="ExternalInput").ap()
    wT_d = nc.dram_tensor("wT", (D, NLOC), f32r, kind="ExternalInput").ap()
    vals_d = nc.dram_tensor("vals", (B, CANDS), f32, kind="ExternalOutput").ap()
    idx_d = nc.dram_tensor("idx", (B, CANDS), u32, kind="ExternalOutput").ap()

    from contextlib import ExitStack

    with tile.TileContext(nc) as tc:
        with ExitStack() as ctx:
            xpool = ctx.enter_context(tc.tile_pool(name="xp", bufs=1))
            wpool = ctx.enter_context(tc.tile_pool(name="wp", bufs=2))
            spool = ctx.enter_context(tc.tile_pool(name="sp", bufs=3))
            ppool = ctx.enter_context(tc.tile_pool(name="pp", bufs=4, space="PSUM"))
            opool = ctx.enter_context(tc.tile_pool(name="op", bufs=3))

            xsb = xpool.tile([128, KT * B], f32r, tag="x")

            for q in range(QTRS):
                wt = wpool.tile([128, KT * NQ], f32r, tag="w")
                for k in range(KT):
                    nc.sync.dma_start(
                        wt[:, k * NQ : (k + 1) * NQ],
                        wT_d[k * 128 : (k + 1) * 128, q * NQ : (q + 1) * NQ],
                    )
                if q == 0:
                    # xT load issued after quarter 0 so the first matmuls'
                    # rhs data is in flight immediately; xT tiles are only
                    # needed progressively per query block.
                    for k in range(KT):
                        nc.sync.dma_start(
                            xsb[:, k * B : (k + 1) * B],
                            xT_d[k * 128 : (k + 1) * 128, :],
                        )
                for b in range(QB):
                    sim = spool.tile([128, NQ], f32, tag="sim")
                    for nt in range(NT):
                        ps = ppool.tile([128, NTILE], f32, tag="ps")
                        for k in range(KT):
                            nc.tensor.matmul(
                                ps[:],
                                xsb[:, k * B + b * 128 : k * B + (b + 1) * 128],
                                wt[:, k * NQ + nt * NTILE : k * NQ + (nt + 1) * NTILE],
                                start=(k == 0),
                                stop=(k == KT - 1),
                            )
                        nc.scalar.copy(sim[:, nt * NTILE : (nt + 1) * NTILE], ps[:])
                    vt = opool.tile([128, CPQ * 8], f32, tag="v")
                    it = opool.tile([128, CPQ * 8], u32, tag="i")
                    for ch in range(CPQ):
                        chunk = sim[:, ch * CHUNK : (ch + 1) * CHUNK]
                        nc.vector.max(vt[:, ch * 8 : (ch + 1) * 8], chunk)
                        nc.vector.max_index(
                            it[:, ch * 8 : (ch + 1) * 8],
                            vt[:, ch * 8 : (ch + 1) * 8],
                            chunk,
                        )
                    nc.sync.dma_start(
                        vals_d[b * 128 : (b + 1) * 128, q * CPQ * 8 : (q + 1) * CPQ * 8],
                        vt[:],
                    )
                    nc.sync.dma_start(
                        idx_d[b * 128 : (b + 1) * 128, q * CPQ * 8 : (q + 1) * CPQ * 8],
                        it[:],
                    )

    nc.compile()
    return nc


def _get_program():
    if "nc" not in _CACHE:
        _CACHE["nc"] = _build_program()
    return _CACHE["nc"]


def _host_merge(x, x_train, y_train, vals, idx):
    """Exact top-200 -> class scores -> ranking, from per-core candidates.

    vals/idx: [NCORES, B, CANDS] candidate values (float32r sims) and
    chunk-local positions. Returns int32 [B, NUM_CLASSES].
    """
    x64 = x.astype(np.float64)
    xt64 = x_train.astype(np.float64)

    # Global column index of each candidate.
    chunk_base = (np.arange(CANDS, dtype=np.int64) // 8) * CHUNK  # [CANDS]
    core_base = (np.arange(NCORES, dtype=np.int64) * NLOC)[:, None, None]
    cols = idx.astype(np.int64) + chunk_base[None, None, :] + core_base
    V = np.concatenate(list(vals), axis=1).astype(np.float64)  # [B, 8*CANDS]
    C = np.concatenate(list(cols), axis=1)  # [B, 8*CANDS]
    M = V.shape[1]

    # Approximate threshold from float32r values.
    t0 = np.partition(V, M - KNN_K, axis=1)[:, M - KNN_K]  # [B]

    # Exact recompute of all candidates within SLACK of the threshold.
    band = np.abs(V - t0[:, None]) <= SLACK
    bq, bj = np.nonzero(band)
    if bq.size:
        bc = C[bq, bj]
        exact = np.einsum("nd,nd->n", x64[bq], xt64[bc])
        V[bq, bj] = exact

    # Top-200 among candidates (ties -> lowest global column, like
    # jax.lax.top_k which prefers lower indices on equal values).
    sel = np.argpartition(-V, KNN_K - 1, axis=1)[:, :KNN_K]
    rows = np.arange(B)[:, None]
    sel_v = V[rows, sel]
    sel_c = C[rows, sel]

    # Exactness checks per query:
    #  (a) tie at the boundary that argpartition may have split arbitrarily
    #  (b) some chunk may have had >8 entries above the threshold
    vmin = sel_v.min(axis=1)  # the 200th largest per row
    tie_rows = (V == vmin[:, None]).sum(axis=1) != (sel_v == vmin[:, None]).sum(axis=1)
    v8 = vals[:, :, 7::8].astype(np.float64)  # [NCORES, B, CANDS//8] chunk 8th-maxes
    flag_rows = (v8 >= (vmin[None, :, None] - SLACK)).any(axis=(0, 2))
    bad = np.nonzero(tie_rows | flag_rows)[0]
    LAST_INFO["fallback_rows"] = int(bad.size)
    for q in bad:
        sims = xt64 @ x64[q]  # [N] exact
        order = np.lexsort((np.arange(N), -sims))[:KNN_K]
        sel_c[q] = order
        sel_v[q] = sims[order]

    # Safety net: weights are exp(sim/0.1) in fp32; for this regime all
    # selected sims are >> 9 so exp overflows to inf exactly like the
    # reference. If that ever failed, recompute selected sims exactly so
    # finite weights match fp32 reference closely.
    if sel_v.min() < 20.0:
        eq, ej = np.nonzero(sel_v < 20.0)
        ec = sel_c[eq, ej]
        sel_v[eq, ej] = np.einsum("nd,nd->n", x64[eq], xt64[ec])

    labels = y_train[sel_c.reshape(-1)].reshape(B, KNN_K).astype(np.int64)
    with np.errstate(over="ignore"):
        w = np.exp((sel_v.astype(np.float32)) / np.float32(KNN_T))
    scores = np.zeros((B, NUM_CLASSES), dtype=np.float32)
    np.add.at(
        scores,
        (np.repeat(np.arange(B), KNN_K), labels.ravel()),
        w.ravel(),
    )
    return np.argsort(-scores, axis=1, kind="stable").astype(np.int32)


def kernel(x, x_train, y_train):
    x = np.asarray(x, dtype=np.float32)
    x_train = np.asarray(x_train, dtype=np.float32)
    y_train = np.asarray(y_train)

    nc = _get_program()

    xT = np.ascontiguousarray(x.T)  # [D, B]
    xtrT = np.ascontiguousarray(x_train.T)  # [D, N]
    in_maps = [
        {
            "xT": xT,
            "wT": np.ascontiguousarray(xtrT[:, c * NLOC : (c + 1) * NLOC]),
        }
        for c in range(NCORES)
    ]

    res = run_bass_kernel_spmd(nc, in_maps, core_ids=list(range(NCORES)))
    LAST_INFO["exec_time_ns"] = res.exec_time_ns
    LAST_INFO["results"] = res

    vals = np.stack([res.results[c]["vals"] for c in range(NCORES)])  # [8, B, CANDS]
    idx = np.stack([res.results[c]["idx"] for c in range(NCORES)])

    return _host_merge(x, x_train, y_train, vals, idx)


# revision 5
# speedup vs baseline: 1.3081x; 1.3081x over previous
"""KNN classifier kernel for Trainium2 (8 NeuronCores, Bass/Tile).

Problem (nn_KNNClassifier): given queries x [4096, 512], train bank
x_train [65536, 512], labels y_train [65536] (100 classes), compute for
each query the top-200 neighbors by dot-product similarity, weight them
by exp(sim/0.1), accumulate per-class scores, and return the descending
argsort of class scores -> int32 [4096, 100].

Device strategy (sharding_hint: shard train bank over N across 8 cores):
  - Host reorders x_train columns by class, zero-padding each class to a
    multiple of 256, so every 256-wide column chunk holds one class.
    Each core takes 1/8 of the chunks plus the full query set.
  - Per core: sim = x @ shard^T via float32r matmuls (full PE rate),
    then one DVE max8 per 256-chunk -> top-8 values per (query, chunk).
    Chunk class is known host-side, so no index extraction is needed;
    zero-pad columns yield exact 0.0 values that the host discards.
  - Host gathers 8 * chunks * 8 candidate values per query -- a superset
    of the global top-200 unless a chunk had >8 entries above threshold,
    which is detected (chunk 8th-max >= threshold - slack) and repaired
    by exact recomputation of that chunk (or per-query fallback).
  - float32r is TF32-like (measured |err| <= ~0.022 at K=512); every
    candidate chunk near the top-200 threshold is recomputed exactly on
    host, so the selected top-200 set matches fp32 reference semantics.
  - Final per-class accumulation mimics the reference exactly (fp32 exp
    -> scatter-add -> stable argsort of negated scores).
"""

import os
import sys

for _p in ("/opt/trn_rl_repo",):
    if _p not in sys.path and os.path.isdir(_p):
        sys.path.insert(0, _p)

import numpy as np

import concourse.mybir as mybir
import concourse.tile as tile
from concourse import bacc
from concourse.bass_utils import run_bass_kernel_spmd

# Problem shapes (hardcoded per spec)
B, N, D = 4096, 65536, 512
NUM_CLASSES = 100
KNN_K = 200
KNN_T = 0.1
NCORES = 8

KT = D // 128  # 4 contraction tiles
QB = B // 128  # 32 query blocks of 128
CHUNK = 256  # class-pure chunk width
NTILE = 512  # matmul moving free dim (2 chunks)

SLACK = 0.05  # exact-recompute band around the top-200 threshold
NEG = -1.0e30

_CACHE = {}
LAST_INFO = {}


def _build_program(C, groups):
    """Per-core Bass program: C chunks of 256 columns, streamed in groups
    of `groups[i]` n-tiles (n-tile = 512 cols = 2 chunks)."""
    nc = bacc.Bacc(
        "TRN2", target_bir_lowering=False, debug=False, num_devices=NCORES
    )
    f32 = mybir.dt.float32
    f32r = mybir.dt.float32r

    ncols = C * CHUNK
    cands = C * 8

    xT_d = nc.dram_tensor("xT", (D, B), f32r, kind="ExternalInput").ap()
    wT_d = nc.dram_tensor("wT", (D, ncols), f32r, kind="ExternalInput").ap()
    vals_d = nc.dram_tensor("vals", (B, cands), f32, kind="ExternalOutput").ap()

    from contextlib import ExitStack

    with tile.TileContext(nc) as tc:
        with ExitStack() as ctx:
            xpool = ctx.enter_context(tc.tile_pool(name="xp", bufs=1))
            wpool = ctx.enter_context(tc.tile_pool(name="wp", bufs=2))
            spool = ctx.enter_context(tc.tile_pool(name="sp", bufs=3))
            ppool = ctx.enter_context(tc.tile_pool(name="pp", bufs=2, space="PSUM"))
            opool = ctx.enter_context(tc.tile_pool(name="op", bufs=3))

            xsb = xpool.tile([128, KT * B], f32r, tag="x")

            col0 = 0  # start column of current group
            for gi, gnt in enumerate(groups):
                gcols = gnt * NTILE
                gchunks = gcols // CHUNK
                wt = wpool.tile([128, KT * gcols], f32r, tag="w")
                for k in range(KT):
                    nc.sync.dma_start(
                        wt[:, k * gcols : (k + 1) * gcols],
                        wT_d[k * 128 : (k + 1) * 128, col0 : col0 + gcols],
                    )
                if gi == 0:
                    # xT load after group 0's rhs so first matmul data is
                    # in flight first; xT is consumed progressively.
                    for k in range(KT):
                        nc.sync.dma_start(
                            xsb[:, k * B : (k + 1) * B],
                            xT_d[k * 128 : (k + 1) * 128, :],
                        )
                for b in range(QB):
                    ps = ppool.tile([128, gcols], f32, tag="ps")
                    for nt in range(gnt):
                        for k in range(KT):
                            nc.tensor.matmul(
                                ps[:, nt * NTILE : (nt + 1) * NTILE],
                                xsb[:, k * B + b * 128 : k * B + (b + 1) * 128],
                                wt[:, k * gcols + nt * NTILE : k * gcols + (nt + 1) * NTILE],
                                start=(k == 0),
                                stop=(k == KT - 1),
                            )
                    sim = spool.tile([128, gcols], f32, tag="sim")
                    nc.scalar.copy(sim[:], ps[:])
                    vt = opool.tile([128, gchunks * 8], f32, tag="v")
                    for ch in range(gchunks):
                        nc.vector.max(
                            vt[:, ch * 8 : (ch + 1) * 8],
                            sim[:, ch * CHUNK : (ch + 1) * CHUNK],
                        )
                    nc.sync.dma_start(
                        vals_d[
                            b * 128 : (b + 1) * 128,
                            (col0 // CHUNK) * 8 : (col0 // CHUNK + gchunks) * 8,
                        ],
                        vt[:],
                    )
                col0 += gcols

    nc.compile()
    return nc


def _get_program(C, groups):
    key = (C, tuple(groups))
    if key not in _CACHE:
        _CACHE[key] = _build_program(C, groups)
    return _CACHE[key]


def _plan_layout(y_train):
    """Class-sorted zero-padded column layout.

    Returns (colmap, chunk_class, C, groups):
      colmap: int64 [8*C*CHUNK] -> original x_train row, or -1 for padding
      chunk_class: int64 [8*C] -> class of each global chunk (-1 dummy)
    """
    cnt = np.bincount(y_train, minlength=NUM_CLASSES)
    by_class = np.argsort(y_train, kind="stable")  # rows grouped by class
    cpc = np.maximum((cnt + CHUNK - 1) // CHUNK, 0)  # chunks per class
    total = int(cpc.sum())
    # round chunk count up so chunks/core is even (integral 512 n-tiles)
    T = ((total + 15) // 16) * 16
    C = T // NCORES

    colmap = np.full(T * CHUNK, -1, dtype=np.int64)
    chunk_class = np.full(T, -1, dtype=np.int64)
    pos = 0  # class-group start within by_class
    col = 0
    ch = 0
    for c in range(NUM_CLASSES):
        n = int(cnt[c])
        colmap[col : col + n] = by_class[pos : pos + n]
        nch = int(cpc[c])
        chunk_class[ch : ch + nch] = c
        pos += n
        col += nch * CHUNK
        ch += nch

    nnt = C * CHUNK // NTILE  # n-tiles per core
    groups = [4] * (nnt // 4)
    if nnt % 4:
        groups.append(nnt % 4)
    return colmap, chunk_class, C, groups


def _host_merge(x, x_train, y_train, vals, colmap, chunk_class, C):
    """Exact top-200 -> class scores -> ranking from per-core candidates."""
    x64 = x.astype(np.float64)
    xt64 = x_train.astype(np.float64)
    T = NCORES * C  # global chunk count
    M = T * 8

    V = np.concatenate(list(vals), axis=1).astype(np.float64)  # [B, M]
    V[V == 0.0] = NEG  # zero-pad artifacts (real sims are never exactly 0)

    kth = M - KNN_K
    t0 = np.partition(V, kth, axis=1)[:, kth]  # [B] approx threshold

    # Chunks needing exact recomputation: any candidate within SLACK of
    # the threshold, or chunk 8th-max (possible hidden elements) near it.
    near = V >= (t0[:, None] - SLACK - 0.01)
    lo = V <= (t0[:, None] + SLACK)
    band = near & lo  # candidate needs exact value
    v8 = V.reshape(B, T, 8)[:, :, 7]
    flag = v8 >= (t0[:, None] - SLACK)  # chunk may hide >8 relevant entries
    chunk_band = band.reshape(B, T, 8).any(axis=2) | flag  # [B, T]

    bq, bg = np.nonzero(chunk_band)
    LAST_INFO["recomputed_chunks"] = int(bq.size)
    full_fallback = set()
    if bq.size:
        # Exact sims per (query, chunk) pair, grouped by chunk so each
        # chunk's column matrix is gathered and transposed only once.
        Vr = V.reshape(B, T, 8)
        order = np.argsort(bg, kind="stable")
        bq_s, bg_s = bq[order], bg[order]
        starts = np.searchsorted(bg_s, np.unique(bg_s))
        bounds = list(starts) + [bg_s.size]
        for i in range(len(starts)):
            s, e = bounds[i], bounds[i + 1]
            g = int(bg_s[s])
            qs = bq_s[s:e]
            rows = colmap[g * CHUNK : (g + 1) * CHUNK]
            pad = rows < 0
            Wg = xt64[np.where(pad, 0, rows)].T  # [D, CHUNK]
            exact = x64[qs] @ Wg  # [nq, CHUNK]
            exact[:, pad] = NEG
            thr = t0[qs] - SLACK - 0.005
            nkeep = (exact >= thr[:, None]).sum(axis=1)
            top8 = -np.sort(-exact, axis=1)[:, :8]
            Vr[qs, g] = top8
            for q in qs[nkeep > 8]:
                full_fallback.add(int(q))

    t1 = np.partition(V, kth, axis=1)[:, kth]
    sel = np.argpartition(-V, KNN_K - 1, axis=1)[:, :KNN_K]
    rowix = np.arange(B)[:, None]
    sel_v = V[rowix, sel]

    # Boundary ties -> per-query fallback (argpartition splits arbitrarily)
    vmin = sel_v.min(axis=1)
    tie = (V == vmin[:, None]).sum(axis=1) != (sel_v == vmin[:, None]).sum(axis=1)
    for q in np.nonzero(tie)[0]:
        full_fallback.add(int(q))
    LAST_INFO["fallback_rows"] = len(full_fallback)

    cand_class = np.repeat(chunk_class, 8)  # [M] class per candidate slot
    labels = cand_class[sel]  # [B, K]

    # Pathological guard: if the top-200 threshold ever sits near/below 0,
    # zero-pad dropping could hide real candidates -> recompute those rows.
    for q in np.nonzero(t1 < 1.0)[0]:
        full_fallback.add(int(q))

    scores = np.zeros((B, NUM_CLASSES), dtype=np.float32)
    with np.errstate(over="ignore"):
        w = np.exp(sel_v.astype(np.float32) / np.float32(KNN_T))
    ok = np.ones(B, dtype=bool)
    for q in full_fallback:
        ok[q] = False
    qs = np.nonzero(ok)[0]
    np.add.at(
        scores,
        (np.repeat(qs, KNN_K), labels[qs].ravel()),
        w[qs].ravel(),
    )

    for q in full_fallback:
        sims = xt64 @ x64[q]
        order = np.lexsort((np.arange(N), -sims))[:KNN_K]
        lab = y_train[order]
        with np.errstate(over="ignore"):
            wq = np.exp(sims[order].astype(np.float32) / np.float32(KNN_T))
        np.add.at(scores[q], lab, wq)

    return np.argsort(-scores, axis=1, kind="stable").astype(np.int32)


def kernel(x, x_train, y_train):
    x = np.asarray(x, dtype=np.float32)
    x_train = np.asarray(x_train, dtype=np.float32)
    y_train = np.asarray(y_train).astype(np.int64)

    colmap, chunk_class, C, groups = _plan_layout(y_train)
    nc = _get_program(C, groups)

    ncols_tot = colmap.shape[0]
    xtrP = np.zeros((D, ncols_tot), dtype=np.float32)  # padded, transposed
    real = colmap >= 0
    xtrP[:, real] = x_train.T[:, colmap[real]]

    xT = np.ascontiguousarray(x.T)
    ncols = C * CHUNK
    in_maps = [
        {
            "xT": xT,
            "wT": np.ascontiguousarray(xtrP[:, c * ncols : (c + 1) * ncols]),
        }
        for c in range(NCORES)
    ]

    res = run_bass_kernel_spmd(nc, in_maps, core_ids=list(range(NCORES)))
    LAST_INFO["exec_time_ns"] = res.exec_time_ns
    LAST_INFO["results"] = res

    vals = np.stack([res.results[c]["vals"] for c in range(NCORES)])  # [8, B, C*8]
    return _host_merge(x, x_train, y_train, vals, colmap, chunk_class, C)
